# revision 19
# baseline (speedup 1.0000x reference)
"""Distributed causal attention head on 8 TRN2 NeuronCores.

Problem: B=4, S=4096, D_in=512, D_out=64 causal attention
  K/V/Q = X @ W; scores = Q@K^T (causal, /sqrt(64)); Z = softmax(scores)@V

Sharding: core c = 2*b + h handles batch b, seq-half h.
q-rows are interleaved at 128-row-block granularity (core h owns global
q-blocks {2j+h}), which makes the causal block schedule IDENTICAL on all
cores (SPMD-safe) and balances FLOPs exactly.  Every core loads the full
(transposed) K/V inputs of its batch and projects them locally.

The whole kernel is interleaved at q-chunk granularity so the PE never
idles >3.4us (HAM stays warm) and compute overlaps the input DMA stream:
for each chunk c: DMA xq[c], xk/xv[2c:2c+2] (separate small tiles ->
precise Tile deps), project Q/K/V for just those columns, PE-transpose
the new V blocks, then run the chunk's attention.  Matmul inputs bf16,
psum/softmax f32.  Scores are computed transposed ST[k,q] with KpT
parity-packed so score matmuls run as row-tiled K=64 PAIRS; exp on ACT
in groups of 3 kblocks (scale=1/8 folded, no max-subtraction:
|scores/8| < ~1.5); AV matmuls accumulate Z^T in PSUM with a
ones-column in Vp giving the softmax denominator for free; Z^T is
PE-transposed back to q-major and normalized with a per-partition
reciprocal + tensor_scalar_mul; output is q-major [2048, 64] f32.
"""

import numpy as np
import ml_dtypes

import concourse.bass as bass
import concourse.bacc as bacc
import concourse.mybir as mybir
import concourse.tile as tile

B, S, D, E = 4, 4096, 512, 64
PB = 128                      # partition block
NKB = S // PB                 # 32 k-blocks (global)
NLQ = NKB // 2                # 16 local q-blocks per core
NCH = 4                       # q-chunks of 512 per core
CHW = 512                     # q-chunk width
ND = D // PB                  # 4 d-slices
GRP = 2                       # kblocks per exp group
LAG = 4                       # ST->AV software pipeline depth (groups)
BF16 = mybir.dt.bfloat16
F32 = mybir.dt.float32
NPBF16 = ml_dtypes.bfloat16


def kparity(kb):
    """kblock -> (partition base, chunk idx, col) in parity-packed kpT."""
    return 64 * (kb % 2), kb // 4, PB * ((kb // 2) % 2)


# band kblock m=0..7 of chunk c (kb=8c+m): first valid q sub-block m//2,
# width 512-128*(m//2); the residual triangular/zero mask covers only the
# first 128 computed columns (class = (m%2) - h: 0 tri, >0 zero, <0 keep)
WBAND = [CHW - PB * (m // 2) for m in range(8)]


def build_nc():
    nc = bacc.Bacc(None)

    xq_d = nc.declare_dram_parameter("xq", [D, S // 2], BF16, isOutput=False)
    xk_d = nc.declare_dram_parameter("xk", [D, S], BF16, isOutput=False)
    xv_d = nc.declare_dram_parameter("xv", [D, S], BF16, isOutput=False)
    # weights pre-swizzled on host: [128, d-slice, E] contiguous
    wq_d = nc.declare_dram_parameter("wq", [PB, ND * E], BF16, isOutput=False)
    wk_d = nc.declare_dram_parameter("wk", [PB, ND * E], BF16, isOutput=False)
    wv_d = nc.declare_dram_parameter("wv", [PB, ND * E], BF16, isOutput=False)
    cm_d = nc.declare_dram_parameter("cmask", [PB, 2 * PB], BF16, isOutput=False)
    id_d = nc.declare_dram_parameter("ident", [PB, PB], F32, isOutput=False)
    out_d = nc.declare_dram_parameter("out", [S // 2, E], F32, isOutput=True)

    with tile.TileContext(nc) as tc:
        with tc.tile_pool(name="persist", bufs=1) as pp, \
             tc.tile_pool(name="st_ps", bufs=2, space="PSUM") as stp, \
             tc.tile_pool(name="pj_ps", bufs=2, space="PSUM") as pjp, \
             tc.tile_pool(name="zt_ps", bufs=2, space="PSUM") as ztp, \
             tc.tile_pool(name="work", bufs=2 * LAG + 2) as wp, \
             tc.tile_pool(name="osb", bufs=3) as op:
            # ---- persistent SBUF tiles ----
            wq_sb = pp.tile([PB, ND * E], BF16, name="wq_sb", tag="wq_sb")
            wk_sb = pp.tile([PB, ND * E], BF16, name="wk_sb", tag="wk_sb")
            wv_sb = pp.tile([PB, ND * E], BF16, name="wv_sb", tag="wv_sb")
            mk_sb = pp.tile([PB, 2 * PB], BF16, name="mk_sb", tag="mk_sb")
            idf_sb = pp.tile([PB, PB], F32, name="idf_sb", tag="idf_sb")
            idb_sb = pp.tile([PB, PB], BF16, name="idb_sb", tag="idb_sb")
            # per-half input tiles (one DMA each -> precise, cheap deps)
            xq_sb = [[pp.tile([PB, 2 * CHW], BF16, name=f"xq{d}_{g}", tag=f"xq{d}_{g}")
                      for g in range(2)] for d in range(ND)]
            xk_sb = [[pp.tile([PB, 4 * CHW], BF16, name=f"xk{d}_{g}", tag=f"xk{d}_{g}")
                      for g in range(2)] for d in range(ND)]
            xv_sb = [[pp.tile([PB, 4 * CHW], BF16, name=f"xv{d}_{g}", tag=f"xv{d}_{g}")
                      for g in range(2)] for d in range(ND)]
            # projected tensors, chunked
            qpT = [pp.tile([PB, CHW], BF16, name=f"qpT{c}", tag=f"qpT{c}")
                   for c in range(NCH)]                    # dup both halves
            kpT = [pp.tile([PB, 2 * PB], BF16, name=f"kpT{c}", tag=f"kpT{c}")
                   for c in range(2 * NCH)]                # parity-packed
            vpT = [pp.tile([E, CHW], BF16, name=f"vpT{c}", tag=f"vpT{c}")
                   for c in range(2 * NCH)]
            vp = [pp.tile([PB, E + 1], BF16, name=f"vp{s}", tag=f"vp{s}")
                  for s in range(NKB)]

            # ---- constant DMAs (one each, on the fast sync queue, first) ----
            for w_d, w_sb in ((wq_d, wq_sb), (wk_d, wk_sb), (wv_d, wv_sb)):
                nc.sync.dma_start(out=w_sb[:], in_=w_d[:])
            for s in range(NKB):
                nc.vector.memset(vp[s][:], 1.0)   # ones column prefill

            # PE warmup: lift the HAM clock gate to 2.4GHz before real
            # compute arrives; gated only on the wq DMA.  Scratch output
            # borrows the first st-pool ring slot (write-only).
            warm_ps = stp.tile([E, 256], F32, tag="st")
            for i in range(18):
                nc.tensor.matmul(warm_ps[:], wq_sb[:, 0:E], wq_sb[:, 0:256],
                                 start=True, stop=True)

            def dma_inputs(g):
                """Issue input DMAs for half g: xq cols, xk/xv cols."""
                for d in range(ND):
                    nc.sync.dma_start(
                        out=xq_sb[d][g][:],
                        in_=xq_d[PB * d:PB * (d + 1), 2 * CHW * g:2 * CHW * (g + 1)])
                for d in range(ND):
                    nc.sync.dma_start(
                        out=xk_sb[d][g][:],
                        in_=xk_d[PB * d:PB * (d + 1), 4 * CHW * g:4 * CHW * (g + 1)])
                if g == 0:
                    nc.gpsimd.dma_start(out=idf_sb[:], in_=id_d[:])
                    nc.vector.tensor_copy(idb_sb[:], idf_sb[:])
                    nc.gpsimd.dma_start(out=mk_sb[:], in_=cm_d[:])
                for d in range(ND):
                    nc.sync.dma_start(
                        out=xv_sb[d][g][:],
                        in_=xv_d[PB * d:PB * (d + 1), 4 * CHW * g:4 * CHW * (g + 1)])

            def vtrans(s):
                """PE-transpose one projected-V block to k-major + copy out."""
                vproj(s // 4)
                vt_ps = pjp.tile([PB, E], BF16, tag="pj")
                nc.tensor.transpose(vt_ps[:], vpT[s // 4][:, PB * (s % 4):PB * (s % 4 + 1)],
                                    idb_sb[0:E, 0:E])
                nc.vector.tensor_copy(vp[s][:, 0:E], vt_ps[:])

            def project(c):
                """Project Q chunk c and K/V chunks 2c, 2c+1 (V transposes
                are emitted later, interleaved between ST groups)."""
                g = c // 2
                qof = CHW * (c % 2)
                qp_ps = pjp.tile([E, CHW], F32, tag="pj")
                for d in range(ND):
                    nc.tensor.matmul(qp_ps[:], wq_sb[:, E * d:E * (d + 1)],
                                     xq_sb[d][g][:, qof:qof + CHW],
                                     start=(d == 0), stop=(d == ND - 1))
                nc.vector.tensor_copy(qpT[c][0:E, :], qp_ps[:])
                nc.scalar.copy(qpT[c][E:2 * E, :], qp_ps[:])
                for kc in (2 * c, 2 * c + 1):
                    kof = CHW * (kc % 4)
                    kp_ps = pjp.tile([E, CHW], F32, tag="pj")
                    for d in range(ND):
                        nc.tensor.matmul(kp_ps[:], wk_sb[:, E * d:E * (d + 1)],
                                         xk_sb[d][g][:, kof:kof + CHW],
                                         start=(d == 0), stop=(d == ND - 1))
                    for j in range(4):
                        kb = 4 * kc + j
                        pb, kch, col = kparity(kb)
                        assert kch == kc
                        nc.vector.tensor_copy(kpT[kc][pb:pb + E, col:col + PB],
                                              kp_ps[:, PB * j:PB * (j + 1)])
            vproj_done = set()

            def vproj(kc):
                """Lazily project V chunk kc (called at first vtrans use)."""
                if kc in vproj_done:
                    return
                vproj_done.add(kc)
                kof = CHW * (kc % 4)
                vq_ps = pjp.tile([E, CHW], F32, tag="pj")
                for d in range(ND):
                    nc.tensor.matmul(vq_ps[:], wv_sb[:, E * d:E * (d + 1)],
                                     xv_sb[d][kc // 4][:, kof:kof + CHW],
                                     start=(d == 0), stop=(d == ND - 1))
                nc.vector.tensor_copy(vpT[kc][:], vq_ps[:])

            def st_mm(st_ps, off, w, kb, c):
                pb, kch, col = kparity(kb)
                nc.tensor.matmul(st_ps[:, off:off + w],
                                 kpT[kch][pb:pb + E, col:col + PB],
                                 qpT[c][pb:pb + E, CHW - w:CHW],
                                 start=True, stop=True, tile_position=(pb, 0))

            # prologue: first half's DMA + first chunk's projections
            dma_inputs(0)
            project(0)

            norm_pend = None
            for c in range(NCH):
                nkb = 8 * c + 8
                zt_ps = ztp.tile([E + 1, CHW], F32, tag="zt")
                # groups: full pairs (kb < 8c, width 512) then banded pairs
                groups = [("full", (kb, kb + 1)) for kb in range(0, 8 * c, 2)]
                groups += [("band", (8 * c + m, 8 * c + m + 1))
                           for m in (0, 2, 4, 6)]
                pend = []
                drain_state = {"n": 0}

                def drain_avs(p_et, p_items, nkb=nkb, zt_ps=zt_ps, c=c, ds=drain_state):
                    for kb, off, w in p_items:   # late vtrans, spread out
                        if kb >= 8 * c:
                            vtrans(kb)
                    for kb, off, w in p_items:
                        jj0 = PB * ((kb - 8 * c) // 2) if kb >= 8 * c else 0
                        nc.tensor.matmul(
                            zt_ps[:, jj0:jj0 + w], vp[kb][:],
                            p_et[:, off:off + w],
                            start=(ds["n"] == 0),
                            stop=(ds["n"] == nkb - 1),
                            skip_group_check=True)
                        ds["n"] += 1

                if c == 0:
                    dma_inputs(1)   # stream second half's inputs early
                for kind, kbs in groups:
                    st_ps = stp.tile([PB, GRP * CHW], F32, tag="st")
                    if kind == "full":
                        items = [(kbs[0], 0, CHW), (kbs[1], CHW, CHW)]
                    else:
                        # second item starts at CHW so neither ST output
                        # crosses a PSUM bank line; the [w0, CHW) gap holds
                        # stale PSUM, exp'd but never consumed by AV/mask
                        w0 = WBAND[kbs[0] - 8 * c]
                        w1 = WBAND[kbs[1] - 8 * c]
                        items = [(kbs[0], 0, w0), (kbs[1], CHW, w1)]
                    for kb, off, w in items:
                        st_mm(st_ps, off, w, kb, c)
                    if len(pend) > LAG - 1:
                        drain_avs(*pend.pop(0))
                    et_sb = wp.tile([PB, GRP * CHW], BF16, tag="et")
                    if items[0][2] == CHW:      # contiguous pair
                        gw = CHW + items[1][2]
                        nc.scalar.activation(
                            et_sb[:, :gw], st_ps[:, :gw],
                            mybir.ActivationFunctionType.Exp, scale=0.125)
                    else:                       # banded pair with a gap
                        for kb, off, w in items:
                            nc.scalar.activation(
                                et_sb[:, off:off + w], st_ps[:, off:off + w],
                                mybir.ActivationFunctionType.Exp, scale=0.125)
                    if kind == "band":
                        for kb, off, w in items:
                            mp = PB * ((kb - 8 * c) % 2)
                            nc.vector.tensor_mul(
                                et_sb[:, off:off + PB],
                                et_sb[:, off:off + PB],
                                mk_sb[:, mp:mp + PB])
                    pend.append((et_sb, items))
                for p in pend:
                    drain_avs(*p)
                zs_sb = wp.tile([E + 1, CHW], F32, tag="zs")
                nc.vector.tensor_copy(zs_sb[:], zt_ps[:])
                # project next chunk while exp/AV tail of this chunk drains
                if c + 1 < NCH:
                    project(c + 1)
                # normalize via transpose (denominator = col E); all four
                # transposes write one PSUM tile so they run back-to-back
                # without DVE round-trips between them
                zn_ps = pjp.tile([PB, 4 * PB], F32, tag="pj")
                for j in range(4):
                    nc.tensor.transpose(zn_ps[:, PB * j:PB * j + E + 1],
                                        zs_sb[:, PB * j:PB * (j + 1)],
                                        idf_sb[0:E + 1, 0:E + 1])
                for j in range(4):
                    rc_sb = wp.tile([PB, 1], F32, tag="rc")
                    nc.vector.reciprocal(rc_sb[:], zn_ps[:, PB * j + E:PB * j + E + 1])
                    o_sb = op.tile([PB, E], F32, tag="osb")
                    nc.vector.tensor_scalar_mul(
                        o_sb[:], zn_ps[:, PB * j:PB * j + E], rc_sb[:])
                    q0 = PB * (4 * c + j)
                    nc.gpsimd.dma_start(out=out_d[q0:q0 + PB, :], in_=o_sb[:])
    nc.finalize()
    return nc


def make_core_inputs(key_np, value_np, query_np, Wk, Wv, Wq):
    """Host-side sharding: returns in_maps list of 8 dicts."""
    bf = lambda a: np.ascontiguousarray(a).astype(NPBF16)
    # weights pre-swizzled: w_sw[p, d, e] = W[128d+p, e], flattened [128, 256]
    sw = lambda Wm: bf(np.asarray(Wm).reshape(ND, PB, E).transpose(1, 0, 2)
                       .reshape(PB, ND * E))
    in_maps = []
    for c in range(8):
        b, h = c // 2, c % 2
        qrows = np.concatenate(
            [np.arange(PB * (2 * j + h), PB * (2 * j + h) + PB) for j in range(NLQ)])
        # first-subblock mask for band kb=8c+m: class = (m%2) - h
        #   0 -> triangular, >0 -> zeros, <0 -> keep
        ki = np.arange(PB)[:, None]
        qi = np.arange(PB)[None, :]
        tri = (ki <= qi).astype(np.float32)
        cmask = np.zeros((PB, 2 * PB), np.float32)
        cmask[:, 0:PB] = tri if h == 0 else 1.0         # m even
        cmask[:, PB:2 * PB] = 0.0 if h == 0 else tri    # m odd
        in_maps.append({
            "xq": bf(query_np[b][qrows].T),
            "xk": bf(key_np[b].T),
            "xv": bf(value_np[b].T),
            "wq": sw(Wq), "wk": sw(Wk), "wv": sw(Wv),
            "cmask": bf(cmask),
            "ident": np.eye(PB, dtype=np.float32),
        })
    return in_maps


def assemble_output(results):
    """results: list of 8 dicts with 'out' [2048, 64] f32 -> Z [B,S,E]."""
    Z = np.zeros((B, S, E), dtype=np.float32)
    for c in range(8):
        b, h = c // 2, c % 2
        o = results[c]["out"]  # [2048, E] q-major
        for j in range(NLQ):
            g = 2 * j + h
            Z[b, PB * g:PB * (g + 1), :] = o[PB * j:PB * (j + 1), :]
    return Z


def kernel(key_inputs, value_inputs, query_inputs, Wk, Wv, Wq):
    from concourse.bass_utils import run_bass_kernel_spmd
    nc = build_nc()
    in_maps = make_core_inputs(np.asarray(key_inputs), np.asarray(value_inputs),
                               np.asarray(query_inputs), np.asarray(Wk),
                               np.asarray(Wv), np.asarray(Wq))
    res = run_bass_kernel_spmd(nc, in_maps, core_ids=list(range(8)))
    return assemble_output(res.results)



# revision 20
# speedup vs baseline: 1.0452x; 1.0452x over previous
"""Distributed causal attention head on 8 TRN2 NeuronCores.

Problem: B=4, S=4096, D_in=512, D_out=64 causal attention
  K/V/Q = X @ W; scores = Q@K^T (causal, /sqrt(64)); Z = softmax(scores)@V

Sharding: core c = 2*b + h handles batch b, seq-half h.
q-rows are interleaved at 128-row-block granularity (core h owns global
q-blocks {2j+h}), which makes the causal block schedule IDENTICAL on all
cores (SPMD-safe) and balances FLOPs exactly.  Every core loads the full
(transposed) K/V inputs of its batch and projects them locally.

The whole kernel is interleaved at q-chunk granularity so the PE never
idles >3.4us (HAM stays warm) and compute overlaps the input DMA stream:
for each chunk c: DMA xq[c], xk/xv[2c:2c+2] (separate small tiles ->
precise Tile deps), project Q/K/V for just those columns, PE-transpose
the new V blocks, then run the chunk's attention.  Matmul inputs bf16,
psum/softmax f32.  Scores are computed transposed ST[k,q] with KpT
parity-packed so score matmuls run as row-tiled K=64 PAIRS; exp on ACT
in groups of 3 kblocks (scale=1/8 folded, no max-subtraction:
|scores/8| < ~1.5); AV matmuls accumulate Z^T in PSUM with a
ones-column in Vp giving the softmax denominator for free; Z^T is
PE-transposed back to q-major and normalized with a per-partition
reciprocal + tensor_scalar_mul; output is q-major [2048, 64] f32.
"""

import numpy as np
import ml_dtypes

import concourse.bass as bass
import concourse.bacc as bacc
import concourse.mybir as mybir
import concourse.tile as tile

B, S, D, E = 4, 4096, 512, 64
PB = 128                      # partition block
NKB = S // PB                 # 32 k-blocks (global)
NLQ = NKB // 2                # 16 local q-blocks per core
NCH = 4                       # q-chunks of 512 per core
CHW = 512                     # q-chunk width
ND = D // PB                  # 4 d-slices
GRP = 2                       # kblocks per exp group
LAG = 4                       # ST->AV software pipeline depth (groups)
BF16 = mybir.dt.bfloat16
F32 = mybir.dt.float32
NPBF16 = ml_dtypes.bfloat16


def kparity(kb):
    """kblock -> (partition base, chunk idx, col) in parity-packed kpT."""
    return 64 * (kb % 2), kb // 4, PB * ((kb // 2) % 2)


# band kblock m=0..7 of chunk c (kb=8c+m): first valid q sub-block m//2,
# width 512-128*(m//2); the residual triangular/zero mask covers only the
# first 128 computed columns (class = (m%2) - h: 0 tri, >0 zero, <0 keep)
WBAND = [CHW - PB * (m // 2) for m in range(8)]


def build_nc():
    nc = bacc.Bacc(None)

    xq_d = nc.declare_dram_parameter("xq", [D, S // 2], BF16, isOutput=False)
    xk_d = nc.declare_dram_parameter("xk", [D, S], BF16, isOutput=False)
    xv_d = nc.declare_dram_parameter("xv", [D, S], BF16, isOutput=False)
    # weights pre-swizzled on host: [128, d-slice, E] contiguous
    wq_d = nc.declare_dram_parameter("wq", [PB, ND * E], BF16, isOutput=False)
    wk_d = nc.declare_dram_parameter("wk", [PB, ND * E], BF16, isOutput=False)
    wv_d = nc.declare_dram_parameter("wv", [PB, ND * E], BF16, isOutput=False)
    cm_d = nc.declare_dram_parameter("cmask", [PB, 2 * PB], BF16, isOutput=False)
    id_d = nc.declare_dram_parameter("ident", [PB, PB], F32, isOutput=False)
    out_d = nc.declare_dram_parameter("out", [S // 2, E], F32, isOutput=True)

    with tile.TileContext(nc) as tc:
        with tc.tile_pool(name="persist", bufs=1) as pp, \
             tc.tile_pool(name="st_ps", bufs=2, space="PSUM") as stp, \
             tc.tile_pool(name="pj_ps", bufs=2, space="PSUM") as pjp, \
             tc.tile_pool(name="zt_ps", bufs=2, space="PSUM") as ztp, \
             tc.tile_pool(name="work", bufs=2 * LAG + 2) as wp, \
             tc.tile_pool(name="osb", bufs=3) as op:
            # ---- persistent SBUF tiles ----
            wq_sb = pp.tile([PB, ND * E], BF16, name="wq_sb", tag="wq_sb")
            wk_sb = pp.tile([PB, ND * E], BF16, name="wk_sb", tag="wk_sb")
            wv_sb = pp.tile([PB, ND * E], BF16, name="wv_sb", tag="wv_sb")
            mk_sb = pp.tile([PB, 2 * PB], BF16, name="mk_sb", tag="mk_sb")
            idf_sb = pp.tile([PB, PB], F32, name="idf_sb", tag="idf_sb")
            idb_sb = pp.tile([PB, PB], BF16, name="idb_sb", tag="idb_sb")
            # per-half input tiles (one DMA each -> precise, cheap deps)
            xq_sb = [[pp.tile([PB, 2 * CHW], BF16, name=f"xq{d}_{g}", tag=f"xq{d}_{g}")
                      for g in range(2)] for d in range(ND)]
            xk_sb = [[pp.tile([PB, 4 * CHW], BF16, name=f"xk{d}_{g}", tag=f"xk{d}_{g}")
                      for g in range(2)] for d in range(ND)]
            xv_sb = [[pp.tile([PB, 4 * CHW], BF16, name=f"xv{d}_{g}", tag=f"xv{d}_{g}")
                      for g in range(2)] for d in range(ND)]
            # projected tensors, chunked
            qpT = [pp.tile([PB, CHW], BF16, name=f"qpT{c}", tag=f"qpT{c}")
                   for c in range(NCH)]                    # dup both halves
            kpT = [pp.tile([PB, 2 * PB], BF16, name=f"kpT{c}", tag=f"kpT{c}")
                   for c in range(2 * NCH)]                # parity-packed
            vpT = [pp.tile([E, CHW], BF16, name=f"vpT{c}", tag=f"vpT{c}")
                   for c in range(2 * NCH)]
            vp = [pp.tile([PB, E + 1], BF16, name=f"vp{s}", tag=f"vp{s}")
                  for s in range(NKB)]

            # ---- input DMA stream: one sync (HWDGE) queue, need-ordered,
            # xk/xv at kc-pair granularity so the K/V pipeline unblocks as
            # early as possible.  Constants ride the gpsimd queue.
            for s in range(NKB):
                nc.vector.memset(vp[s][:], 1.0)   # ones column prefill

            def dma_xq(g):
                for d in range(ND):
                    nc.sync.dma_start(
                        out=xq_sb[d][g][:],
                        in_=xq_d[PB * d:PB * (d + 1), 2 * CHW * g:2 * CHW * (g + 1)])

            def dma_xkv(x_d, x_sb, g, half):
                o = 4 * CHW * g + 2 * CHW * half
                for d in range(ND):
                    nc.sync.dma_start(
                        out=x_sb[d][g][:, 2 * CHW * half:2 * CHW * (half + 1)],
                        in_=x_d[PB * d:PB * (d + 1), o:o + 2 * CHW])

            nc.sync.dma_start(out=wq_sb[:], in_=wq_d[:])
            dma_xq(0)
            nc.sync.dma_start(out=wk_sb[:], in_=wk_d[:])
            dma_xkv(xk_d, xk_sb, 0, 0)
            nc.sync.dma_start(out=wv_sb[:], in_=wv_d[:])
            nc.gpsimd.dma_start(out=idf_sb[:], in_=id_d[:])
            nc.vector.tensor_copy(idb_sb[:], idf_sb[:])
            nc.gpsimd.dma_start(out=mk_sb[:], in_=cm_d[:])
            dma_xkv(xv_d, xv_sb, 0, 0)
            dma_xkv(xk_d, xk_sb, 0, 1)
            dma_xkv(xv_d, xv_sb, 0, 1)
            dma_xq(1)
            dma_xkv(xk_d, xk_sb, 1, 0)
            dma_xkv(xv_d, xv_sb, 1, 0)
            dma_xkv(xk_d, xk_sb, 1, 1)
            dma_xkv(xv_d, xv_sb, 1, 1)

            def dma_inputs(g):
                pass

            def vtrans(s):
                """PE-transpose one projected-V block to k-major + copy out."""
                vproj(s // 4)
                vt_ps = pjp.tile([PB, E], BF16, tag="pj")
                nc.tensor.transpose(vt_ps[:], vpT[s // 4][:, PB * (s % 4):PB * (s % 4 + 1)],
                                    idb_sb[0:E, 0:E])
                nc.vector.tensor_copy(vp[s][:, 0:E], vt_ps[:])

            def project(c):
                """Project Q chunk c and K/V chunks 2c, 2c+1 (V transposes
                are emitted later, interleaved between ST groups)."""
                g = c // 2
                qof = CHW * (c % 2)
                qp_ps = pjp.tile([E, CHW], F32, tag="pj")
                for d in range(ND):
                    nc.tensor.matmul(qp_ps[:], wq_sb[:, E * d:E * (d + 1)],
                                     xq_sb[d][g][:, qof:qof + CHW],
                                     start=(d == 0), stop=(d == ND - 1))
                nc.vector.tensor_copy(qpT[c][0:E, :], qp_ps[:])
                nc.scalar.copy(qpT[c][E:2 * E, :], qp_ps[:])
                for kc in (2 * c, 2 * c + 1):
                    kof = CHW * (kc % 4)
                    kp_ps = pjp.tile([E, CHW], F32, tag="pj")
                    for d in range(ND):
                        nc.tensor.matmul(kp_ps[:], wk_sb[:, E * d:E * (d + 1)],
                                         xk_sb[d][g][:, kof:kof + CHW],
                                         start=(d == 0), stop=(d == ND - 1))
                    for j in range(4):
                        kb = 4 * kc + j
                        pb, kch, col = kparity(kb)
                        assert kch == kc
                        nc.vector.tensor_copy(kpT[kc][pb:pb + E, col:col + PB],
                                              kp_ps[:, PB * j:PB * (j + 1)])
            vproj_done = set()

            def vproj(kc):
                """Lazily project V chunk kc (called at first vtrans use)."""
                if kc in vproj_done:
                    return
                vproj_done.add(kc)
                kof = CHW * (kc % 4)
                vq_ps = pjp.tile([E, CHW], F32, tag="pj")
                for d in range(ND):
                    nc.tensor.matmul(vq_ps[:], wv_sb[:, E * d:E * (d + 1)],
                                     xv_sb[d][kc // 4][:, kof:kof + CHW],
                                     start=(d == 0), stop=(d == ND - 1))
                nc.vector.tensor_copy(vpT[kc][:], vq_ps[:])

            def st_mm(st_ps, off, w, kb, c):
                pb, kch, col = kparity(kb)
                nc.tensor.matmul(st_ps[:, off:off + w],
                                 kpT[kch][pb:pb + E, col:col + PB],
                                 qpT[c][pb:pb + E, CHW - w:CHW],
                                 start=True, stop=True, tile_position=(pb, 0))

            # prologue: first half's DMA + first chunk's projections
            dma_inputs(0)
            project(0)

            norm_pend = None
            for c in range(NCH):
                nkb = 8 * c + 8
                zt_ps = ztp.tile([E + 1, CHW], F32, tag="zt")
                # groups: full pairs (kb < 8c, width 512) then banded pairs
                groups = [("full", (kb, kb + 1)) for kb in range(0, 8 * c, 2)]
                groups += [("band", (8 * c + m, 8 * c + m + 1))
                           for m in (0, 2, 4, 6)]
                pend = []
                drain_state = {"n": 0}

                def drain_avs(p_et, p_items, nkb=nkb, zt_ps=zt_ps, c=c, ds=drain_state):
                    for kb, off, w in p_items:   # late vtrans, spread out
                        if kb >= 8 * c:
                            vtrans(kb)
                    for kb, off, w in p_items:
                        jj0 = PB * ((kb - 8 * c) // 2) if kb >= 8 * c else 0
                        nc.tensor.matmul(
                            zt_ps[:, jj0:jj0 + w], vp[kb][:],
                            p_et[:, off:off + w],
                            start=(ds["n"] == 0),
                            stop=(ds["n"] == nkb - 1),
                            skip_group_check=True)
                        ds["n"] += 1

                if c == 0:
                    dma_inputs(1)   # stream second half's inputs early
                for kind, kbs in groups:
                    st_ps = stp.tile([PB, GRP * CHW], F32, tag="st")
                    if kind == "full":
                        items = [(kbs[0], 0, CHW), (kbs[1], CHW, CHW)]
                    else:
                        # second item starts at CHW so neither ST output
                        # crosses a PSUM bank line; the [w0, CHW) gap holds
                        # stale PSUM, exp'd but never consumed by AV/mask
                        w0 = WBAND[kbs[0] - 8 * c]
                        w1 = WBAND[kbs[1] - 8 * c]
                        items = [(kbs[0], 0, w0), (kbs[1], CHW, w1)]
                    for kb, off, w in items:
                        st_mm(st_ps, off, w, kb, c)
                    if len(pend) > LAG - 1:
                        drain_avs(*pend.pop(0))
                    et_sb = wp.tile([PB, GRP * CHW], BF16, tag="et")
                    if items[0][2] == CHW:      # contiguous pair
                        gw = CHW + items[1][2]
                        nc.scalar.activation(
                            et_sb[:, :gw], st_ps[:, :gw],
                            mybir.ActivationFunctionType.Exp, scale=0.125)
                    else:                       # banded pair with a gap
                        for kb, off, w in items:
                            nc.scalar.activation(
                                et_sb[:, off:off + w], st_ps[:, off:off + w],
                                mybir.ActivationFunctionType.Exp, scale=0.125)
                    if kind == "band":
                        for kb, off, w in items:
                            mp = PB * ((kb - 8 * c) % 2)
                            nc.vector.tensor_mul(
                                et_sb[:, off:off + PB],
                                et_sb[:, off:off + PB],
                                mk_sb[:, mp:mp + PB])
                    pend.append((et_sb, items))
                for p in pend:
                    drain_avs(*p)
                zs_sb = wp.tile([E + 1, CHW], F32, tag="zs")
                nc.vector.tensor_copy(zs_sb[:], zt_ps[:])
                # project next chunk while exp/AV tail of this chunk drains
                if c + 1 < NCH:
                    project(c + 1)
                # normalize via transpose (denominator = col E); all four
                # transposes write one PSUM tile so they run back-to-back
                # without DVE round-trips between them
                zn_ps = pjp.tile([PB, 4 * PB], F32, tag="pj")
                for j in range(4):
                    nc.tensor.transpose(zn_ps[:, PB * j:PB * j + E + 1],
                                        zs_sb[:, PB * j:PB * (j + 1)],
                                        idf_sb[0:E + 1, 0:E + 1])
                for j in range(4):
                    rc_sb = wp.tile([PB, 1], F32, tag="rc")
                    nc.vector.reciprocal(rc_sb[:], zn_ps[:, PB * j + E:PB * j + E + 1])
                    o_sb = op.tile([PB, E], F32, tag="osb")
                    nc.vector.tensor_scalar_mul(
                        o_sb[:], zn_ps[:, PB * j:PB * j + E], rc_sb[:])
                    q0 = PB * (4 * c + j)
                    nc.sync.dma_start(out=out_d[q0:q0 + PB, :], in_=o_sb[:])
    nc.finalize()
    return nc


def make_core_inputs(key_np, value_np, query_np, Wk, Wv, Wq):
    """Host-side sharding: returns in_maps list of 8 dicts."""
    bf = lambda a: np.ascontiguousarray(a).astype(NPBF16)
    # weights pre-swizzled: w_sw[p, d, e] = W[128d+p, e], flattened [128, 256]
    sw = lambda Wm: bf(np.asarray(Wm).reshape(ND, PB, E).transpose(1, 0, 2)
                       .reshape(PB, ND * E))
    in_maps = []
    for c in range(8):
        b, h = c // 2, c % 2
        qrows = np.concatenate(
            [np.arange(PB * (2 * j + h), PB * (2 * j + h) + PB) for j in range(NLQ)])
        # first-subblock mask for band kb=8c+m: class = (m%2) - h
        #   0 -> triangular, >0 -> zeros, <0 -> keep
        ki = np.arange(PB)[:, None]
        qi = np.arange(PB)[None, :]
        tri = (ki <= qi).astype(np.float32)
        cmask = np.zeros((PB, 2 * PB), np.float32)
        cmask[:, 0:PB] = tri if h == 0 else 1.0         # m even
        cmask[:, PB:2 * PB] = 0.0 if h == 0 else tri    # m odd
        in_maps.append({
            "xq": bf(query_np[b][qrows].T),
            "xk": bf(key_np[b].T),
            "xv": bf(value_np[b].T),
            "wq": sw(Wq), "wk": sw(Wk), "wv": sw(Wv),
            "cmask": bf(cmask),
            "ident": np.eye(PB, dtype=np.float32),
        })
    return in_maps


def assemble_output(results):
    """results: list of 8 dicts with 'out' [2048, 64] f32 -> Z [B,S,E]."""
    Z = np.zeros((B, S, E), dtype=np.float32)
    for c in range(8):
        b, h = c // 2, c % 2
        o = results[c]["out"]  # [2048, E] q-major
        for j in range(NLQ):
            g = 2 * j + h
            Z[b, PB * g:PB * (g + 1), :] = o[PB * j:PB * (j + 1), :]
    return Z


def kernel(key_inputs, value_inputs, query_inputs, Wk, Wv, Wq):
    from concourse.bass_utils import run_bass_kernel_spmd
    nc = build_nc()
    in_maps = make_core_inputs(np.asarray(key_inputs), np.asarray(value_inputs),
                               np.asarray(query_inputs), np.asarray(Wk),
                               np.asarray(Wv), np.asarray(Wq))
    res = run_bass_kernel_spmd(nc, in_maps, core_ids=list(range(8)))
    return assemble_output(res.results)



# revision 21
# speedup vs baseline: 1.1436x; 1.0941x over previous
"""Distributed causal attention head on 8 TRN2 NeuronCores.

Problem: B=4, S=4096, D_in=512, D_out=64 causal attention
  K/V/Q = X @ W; scores = Q@K^T (causal, /sqrt(64)); Z = softmax(scores)@V

Sharding: core c = 2*b + h handles batch b, seq-half h.
q-rows are interleaved at 128-row-block granularity (core h owns global
q-blocks {2j+h}), which makes the causal block schedule IDENTICAL on all
cores (SPMD-safe) and balances FLOPs exactly.  Every core loads the full
(transposed) K/V inputs of its batch and projects them locally.

The whole kernel is interleaved at q-chunk granularity so the PE never
idles >3.4us (HAM stays warm) and compute overlaps the input DMA stream:
for each chunk c: DMA xq[c], xk/xv[2c:2c+2] (separate small tiles ->
precise Tile deps), project Q/K/V for just those columns, PE-transpose
the new V blocks, then run the chunk's attention.  Matmul inputs bf16,
psum/softmax f32.  Scores are computed transposed ST[k,q] with KpT
parity-packed so score matmuls run as row-tiled K=64 PAIRS; exp on ACT
in groups of 3 kblocks (scale=1/8 folded, no max-subtraction:
|scores/8| < ~1.5); AV matmuls accumulate Z^T in PSUM with a
ones-column in Vp giving the softmax denominator for free; Z^T is
PE-transposed back to q-major and normalized with a per-partition
reciprocal + tensor_scalar_mul; output is q-major [2048, 64] f32.
"""

import numpy as np
import ml_dtypes

import concourse.bass as bass
import concourse.bacc as bacc
import concourse.mybir as mybir
import concourse.tile as tile

B, S, D, E = 4, 4096, 512, 64
PB = 128                      # partition block
NKB = S // PB                 # 32 k-blocks (global)
NLQ = NKB // 2                # 16 local q-blocks per core
NCH = 4                       # q-chunks of 512 per core
CHW = 512                     # q-chunk width
ND = D // PB                  # 4 d-slices
GRP = 2                       # kblocks per exp group
LAG = 4                       # ST->AV software pipeline depth (groups)
BF16 = mybir.dt.bfloat16
F32 = mybir.dt.float32
NPBF16 = ml_dtypes.bfloat16


def kparity(kb):
    """kblock -> (partition base, chunk idx, col) in parity-packed kpT."""
    return 64 * (kb % 2), kb // 4, PB * ((kb // 2) % 2)


# band kblock m=0..7 of chunk c (kb=8c+m): first valid q sub-block m//2,
# width 512-128*(m//2); the residual triangular/zero mask covers only the
# first 128 computed columns (class = (m%2) - h: 0 tri, >0 zero, <0 keep)
WBAND = [CHW - PB * (m // 2) for m in range(8)]


def build_nc():
    nc = bacc.Bacc(None)

    xq_d = nc.declare_dram_parameter("xq", [D, S // 2], BF16, isOutput=False)
    xk_d = nc.declare_dram_parameter("xk", [D, S], BF16, isOutput=False)
    xv_d = nc.declare_dram_parameter("xv", [D, S], BF16, isOutput=False)
    # weights pre-swizzled on host: [128, d-slice, E] contiguous
    wq_d = nc.declare_dram_parameter("wq", [PB, ND * E], BF16, isOutput=False)
    wk_d = nc.declare_dram_parameter("wk", [PB, ND * E], BF16, isOutput=False)
    wv_d = nc.declare_dram_parameter("wv", [PB, ND * E], BF16, isOutput=False)
    cm_d = nc.declare_dram_parameter("cmask", [PB, 2 * PB], BF16, isOutput=False)
    id_d = nc.declare_dram_parameter("ident", [PB, PB], F32, isOutput=False)
    out_d = nc.declare_dram_parameter("out", [S // 2, E], F32, isOutput=True)

    with tile.TileContext(nc) as tc:
        with tc.tile_pool(name="persist", bufs=1) as pp, \
             tc.tile_pool(name="st_ps", bufs=2, space="PSUM") as stp, \
             tc.tile_pool(name="pj_ps", bufs=2, space="PSUM") as pjp, \
             tc.tile_pool(name="zt_ps", bufs=2, space="PSUM") as ztp, \
             tc.tile_pool(name="work", bufs=2 * LAG + 2) as wp, \
             tc.tile_pool(name="osb", bufs=3) as op:
            # ---- persistent SBUF tiles ----
            wq_sb = pp.tile([PB, ND * E], BF16, name="wq_sb", tag="wq_sb")
            wk_sb = pp.tile([PB, ND * E], BF16, name="wk_sb", tag="wk_sb")
            wv_sb = pp.tile([PB, ND * E], BF16, name="wv_sb", tag="wv_sb")
            mk_sb = pp.tile([PB, 2 * PB], BF16, name="mk_sb", tag="mk_sb")
            idf_sb = pp.tile([PB, PB], F32, name="idf_sb", tag="idf_sb")
            idb_sb = pp.tile([PB, PB], BF16, name="idb_sb", tag="idb_sb")
            # per-half input tiles (one DMA each -> precise, cheap deps)
            xq_sb = [[pp.tile([PB, 2 * CHW], BF16, name=f"xq{d}_{g}", tag=f"xq{d}_{g}")
                      for g in range(2)] for d in range(ND)]
            xk_sb = [[pp.tile([PB, 4 * CHW], BF16, name=f"xk{d}_{g}", tag=f"xk{d}_{g}")
                      for g in range(2)] for d in range(ND)]
            xv_sb = [[pp.tile([PB, 4 * CHW], BF16, name=f"xv{d}_{g}", tag=f"xv{d}_{g}")
                      for g in range(2)] for d in range(ND)]
            # projected tensors, chunked
            qpT = [pp.tile([PB, CHW], BF16, name=f"qpT{c}", tag=f"qpT{c}")
                   for c in range(NCH)]                    # dup both halves
            kpT = [pp.tile([PB, 2 * PB], BF16, name=f"kpT{c}", tag=f"kpT{c}")
                   for c in range(2 * NCH)]                # parity-packed
            vpT = [pp.tile([E, CHW], BF16, name=f"vpT{c}", tag=f"vpT{c}")
                   for c in range(2 * NCH)]
            vp = [pp.tile([PB, E + 1], BF16, name=f"vp{s}", tag=f"vp{s}")
                  for s in range(NKB)]

            # ---- input DMA stream: one sync (HWDGE) queue, need-ordered,
            # xk/xv at kc-pair granularity so the K/V pipeline unblocks as
            # early as possible.  Constants ride the gpsimd queue.
            for s in range(NKB):
                nc.vector.memset(vp[s][:], 1.0)   # ones column prefill

            def dma_xq(g):
                for d in range(ND):
                    nc.sync.dma_start(
                        out=xq_sb[d][g][:],
                        in_=xq_d[PB * d:PB * (d + 1), 2 * CHW * g:2 * CHW * (g + 1)])

            def dma_xkv(x_d, x_sb, g, half):
                o = 4 * CHW * g + 2 * CHW * half
                for d in range(ND):
                    nc.sync.dma_start(
                        out=x_sb[d][g][:, 2 * CHW * half:2 * CHW * (half + 1)],
                        in_=x_d[PB * d:PB * (d + 1), o:o + 2 * CHW])

            nc.sync.dma_start(out=wq_sb[:], in_=wq_d[:])
            dma_xq(0)
            nc.sync.dma_start(out=wk_sb[:], in_=wk_d[:])
            dma_xkv(xk_d, xk_sb, 0, 0)
            nc.sync.dma_start(out=wv_sb[:], in_=wv_d[:])
            nc.gpsimd.dma_start(out=idf_sb[:], in_=id_d[:])
            nc.vector.tensor_copy(idb_sb[:], idf_sb[:])
            nc.gpsimd.dma_start(out=mk_sb[:], in_=cm_d[:])
            dma_xkv(xv_d, xv_sb, 0, 0)
            dma_xkv(xk_d, xk_sb, 0, 1)
            dma_xkv(xv_d, xv_sb, 0, 1)
            dma_xq(1)
            dma_xkv(xk_d, xk_sb, 1, 0)
            dma_xkv(xv_d, xv_sb, 1, 0)
            dma_xkv(xk_d, xk_sb, 1, 1)
            dma_xkv(xv_d, xv_sb, 1, 1)

            def dma_inputs(g):
                pass

            def vtrans(s):
                """PE-transpose one projected-V block to k-major + copy out."""
                vproj(s // 4)
                vt_ps = pjp.tile([PB, E], BF16, tag="pj")
                nc.tensor.transpose(vt_ps[:], vpT[s // 4][:, PB * (s % 4):PB * (s % 4 + 1)],
                                    idb_sb[0:E, 0:E])
                nc.vector.tensor_copy(vp[s][:, 0:E], vt_ps[:])

            def project(c):
                """Project Q chunk c and K/V chunks 2c, 2c+1 (V transposes
                are emitted later, interleaved between ST groups)."""
                g = c // 2
                qof = CHW * (c % 2)
                qp_ps = pjp.tile([E, CHW], F32, tag="pj")
                for d in range(ND):
                    nc.tensor.matmul(qp_ps[:], wq_sb[:, E * d:E * (d + 1)],
                                     xq_sb[d][g][:, qof:qof + CHW],
                                     start=(d == 0), stop=(d == ND - 1))
                nc.vector.tensor_copy(qpT[c][0:E, :], qp_ps[:])
                nc.scalar.copy(qpT[c][E:2 * E, :], qp_ps[:])
                for kc in (2 * c, 2 * c + 1):
                    kof = CHW * (kc % 4)
                    kp_ps = pjp.tile([E, CHW], F32, tag="pj")
                    for d in range(ND):
                        nc.tensor.matmul(kp_ps[:], wk_sb[:, E * d:E * (d + 1)],
                                         xk_sb[d][g][:, kof:kof + CHW],
                                         start=(d == 0), stop=(d == ND - 1))
                    for j in range(4):
                        kb = 4 * kc + j
                        pb, kch, col = kparity(kb)
                        assert kch == kc
                        nc.vector.tensor_copy(kpT[kc][pb:pb + E, col:col + PB],
                                              kp_ps[:, PB * j:PB * (j + 1)])
            vproj_done = set()
            vtrans_done = set()

            def prep_v(c):
                """Eagerly project + transpose V for chunk c's new kblocks
                (kbs 8c..8c+7) so the chunk's AV drains never wait on the
                vproj->cast->transpose->copy chain."""
                for kb in range(8 * c, 8 * c + 8):
                    if kb not in vtrans_done:
                        vtrans_done.add(kb)
                        vtrans(kb)

            def vproj(kc):
                """Lazily project V chunk kc (called at first vtrans use)."""
                if kc in vproj_done:
                    return
                vproj_done.add(kc)
                kof = CHW * (kc % 4)
                vq_ps = pjp.tile([E, CHW], F32, tag="pj")
                for d in range(ND):
                    nc.tensor.matmul(vq_ps[:], wv_sb[:, E * d:E * (d + 1)],
                                     xv_sb[d][kc // 4][:, kof:kof + CHW],
                                     start=(d == 0), stop=(d == ND - 1))
                nc.vector.tensor_copy(vpT[kc][:], vq_ps[:])

            def st_mm(st_ps, off, w, kb, c):
                pb, kch, col = kparity(kb)
                nc.tensor.matmul(st_ps[:, off:off + w],
                                 kpT[kch][pb:pb + E, col:col + PB],
                                 qpT[c][pb:pb + E, CHW - w:CHW],
                                 start=True, stop=True, tile_position=(pb, 0))

            # prologue: first half's DMA + first chunk's projections
            dma_inputs(0)
            project(0)

            norm_pend = None
            for c in range(NCH):
                nkb = 8 * c + 8
                zt_ps = ztp.tile([E + 1, CHW], F32, tag="zt")
                # groups: full pairs (kb < 8c, width 512) then banded pairs
                groups = [("full", (kb, kb + 1)) for kb in range(0, 8 * c, 2)]
                groups += [("band", (8 * c + m, 8 * c + m + 1))
                           for m in (0, 2, 4, 6)]
                pend = []
                drain_state = {"n": 0}

                def drain_avs(p_et, p_items, nkb=nkb, zt_ps=zt_ps, c=c, ds=drain_state):
                    for kb, off, w in p_items:
                        if kb >= 8 * c and kb not in vtrans_done:
                            vtrans_done.add(kb)
                            vtrans(kb)
                    for kb, off, w in p_items:
                        jj0 = PB * ((kb - 8 * c) // 2) if kb >= 8 * c else 0
                        nc.tensor.matmul(
                            zt_ps[:, jj0:jj0 + w], vp[kb][:],
                            p_et[:, off:off + w],
                            start=(ds["n"] == 0),
                            stop=(ds["n"] == nkb - 1),
                            skip_group_check=True)
                        ds["n"] += 1

                if c == 0:
                    dma_inputs(1)   # stream second half's inputs early
                for kind, kbs in groups:
                    st_ps = stp.tile([PB, GRP * CHW], F32, tag="st")
                    if kind == "full":
                        items = [(kbs[0], 0, CHW), (kbs[1], CHW, CHW)]
                    else:
                        # second item starts at CHW so neither ST output
                        # crosses a PSUM bank line; the [w0, CHW) gap holds
                        # stale PSUM, exp'd but never consumed by AV/mask
                        w0 = WBAND[kbs[0] - 8 * c]
                        w1 = WBAND[kbs[1] - 8 * c]
                        items = [(kbs[0], 0, w0), (kbs[1], CHW, w1)]
                    for kb, off, w in items:
                        st_mm(st_ps, off, w, kb, c)
                    if len(pend) > LAG - 1:
                        drain_avs(*pend.pop(0))
                    et_sb = wp.tile([PB, GRP * CHW], BF16, tag="et")
                    if items[0][2] == CHW:      # contiguous pair
                        gw = CHW + items[1][2]
                        nc.scalar.activation(
                            et_sb[:, :gw], st_ps[:, :gw],
                            mybir.ActivationFunctionType.Exp, scale=0.125)
                    else:                       # banded pair with a gap
                        for kb, off, w in items:
                            nc.scalar.activation(
                                et_sb[:, off:off + w], st_ps[:, off:off + w],
                                mybir.ActivationFunctionType.Exp, scale=0.125)
                    if kind == "band":
                        for kb, off, w in items:
                            mp = PB * ((kb - 8 * c) % 2)
                            nc.vector.tensor_mul(
                                et_sb[:, off:off + PB],
                                et_sb[:, off:off + PB],
                                mk_sb[:, mp:mp + PB])
                    pend.append((et_sb, items))
                for p in pend:
                    drain_avs(*p)
                zs_sb = wp.tile([E + 1, CHW], F32, tag="zs")
                nc.vector.tensor_copy(zs_sb[:], zt_ps[:])
                # project next chunk while exp/AV tail of this chunk drains
                if c + 1 < NCH:
                    project(c + 1)
                    prep_v(c + 1)
                # normalize via transpose (denominator = col E); all four
                # transposes write one PSUM tile so they run back-to-back
                # without DVE round-trips between them
                zn_ps = pjp.tile([PB, 4 * PB], F32, tag="pj")
                for j in range(4):
                    nc.tensor.transpose(zn_ps[:, PB * j:PB * j + E + 1],
                                        zs_sb[:, PB * j:PB * (j + 1)],
                                        idf_sb[0:E + 1, 0:E + 1])
                for j in range(4):
                    rc_sb = wp.tile([PB, 1], F32, tag="rc")
                    nc.vector.reciprocal(rc_sb[:], zn_ps[:, PB * j + E:PB * j + E + 1])
                    o_sb = op.tile([PB, E], F32, tag="osb")
                    nc.vector.tensor_scalar_mul(
                        o_sb[:], zn_ps[:, PB * j:PB * j + E], rc_sb[:])
                    q0 = PB * (4 * c + j)
                    nc.sync.dma_start(out=out_d[q0:q0 + PB, :], in_=o_sb[:])
    nc.finalize()
    return nc


def make_core_inputs(key_np, value_np, query_np, Wk, Wv, Wq):
    """Host-side sharding: returns in_maps list of 8 dicts."""
    bf = lambda a: np.ascontiguousarray(a).astype(NPBF16)
    # weights pre-swizzled: w_sw[p, d, e] = W[128d+p, e], flattened [128, 256]
    sw = lambda Wm: bf(np.asarray(Wm).reshape(ND, PB, E).transpose(1, 0, 2)
                       .reshape(PB, ND * E))
    in_maps = []
    for c in range(8):
        b, h = c // 2, c % 2
        qrows = np.concatenate(
            [np.arange(PB * (2 * j + h), PB * (2 * j + h) + PB) for j in range(NLQ)])
        # first-subblock mask for band kb=8c+m: class = (m%2) - h
        #   0 -> triangular, >0 -> zeros, <0 -> keep
        ki = np.arange(PB)[:, None]
        qi = np.arange(PB)[None, :]
        tri = (ki <= qi).astype(np.float32)
        cmask = np.zeros((PB, 2 * PB), np.float32)
        cmask[:, 0:PB] = tri if h == 0 else 1.0         # m even
        cmask[:, PB:2 * PB] = 0.0 if h == 0 else tri    # m odd
        in_maps.append({
            "xq": bf(query_np[b][qrows].T),
            "xk": bf(key_np[b].T),
            "xv": bf(value_np[b].T),
            "wq": sw(Wq), "wk": sw(Wk), "wv": sw(Wv),
            "cmask": bf(cmask),
            "ident": np.eye(PB, dtype=np.float32),
        })
    return in_maps


def assemble_output(results):
    """results: list of 8 dicts with 'out' [2048, 64] f32 -> Z [B,S,E]."""
    Z = np.zeros((B, S, E), dtype=np.float32)
    for c in range(8):
        b, h = c // 2, c % 2
        o = results[c]["out"]  # [2048, E] q-major
        for j in range(NLQ):
            g = 2 * j + h
            Z[b, PB * g:PB * (g + 1), :] = o[PB * j:PB * (j + 1), :]
    return Z


def kernel(key_inputs, value_inputs, query_inputs, Wk, Wv, Wq):
    from concourse.bass_utils import run_bass_kernel_spmd
    nc = build_nc()
    in_maps = make_core_inputs(np.asarray(key_inputs), np.asarray(value_inputs),
                               np.asarray(query_inputs), np.asarray(Wk),
                               np.asarray(Wv), np.asarray(Wq))
    res = run_bass_kernel_spmd(nc, in_maps, core_ids=list(range(8)))
    return assemble_output(res.results)

